# revision 19
# baseline (speedup 1.0000x reference)
"""DeepFM forward kernel for 8 Trainium2 NeuronCores (Bass/Tile), v4.

Single-phase design (v3 was two phases; its phase B alone cost 27us):

  - Data-parallel over batch: B=16384 -> 2048 rows/core; tables+weights
    replicated.
  - Host builds per-field tables [size_f, 256] bf16 whose rows are
    [emb(128) | fc | rowsq | rowsum | 0-pad]: ONE transposed SWDGE gather
    per (field, j-tile) yields the feature-major emb tile in block 0 AND
    the per-row scalars (fc value, sum emb^2, sum emb) on partitions
    0/1/2 of block 1.  This kills v3's 64 per-chunk fc indirect DMAs
    (~66us of Q7 fixed overhead) and all FM square work.
  - A tiny [128,3] selector matmul per (field, j) accumulates
    (lin, rowsumsq, rowsum) stats into PSUM; the FM global scalar partial
    is reduced on DVE.  The cross-core sum uses an in-kernel AllGather of
    the 8 scalars (floor ~5us, fully hidden under the MLP) -> no second
    kernel launch.
  - MLP in fp8 DoubleRow as v3 (weights host-packed [ki,(g ko),m]); all
    activations are per-mt [128,512] Relu+bias on scalar/vector engines;
    bf16->fp8 input casts run on scalar early and gpsimd (SBUF-only
    engine) once the 16 gathers have drained its queue.
  - PE work is one continuous stream (stats j -> L1 j -> L2 j-1 -> ...)
    to hold the HAM throttle at full clock (2.4GHz needs >=4us of
    uninterrupted PE activity).
  - Final y = sigmoid(ypre + lin + S) on-device; output [1, 2048] f32.
"""

import os
import numpy as np
import ml_dtypes

# ---- problem constants (hardcoded; kernel.py must be self-contained) ----
TOTAL = 38279
CAT_SIZES = [31360, 6807, 18, 94]
EMB = 128
F = 4
B = 16384
N_CORES = 8
P = 128
NB = 512                       # matmul moving width (batch columns)
ROWW = 256                     # padded table row width (bf16 elems)
OFFSETS_NP = np.array([0, 31360, 38167, 38185], dtype=np.int32)

_build_cache = {}


def _build_main(b_loc, n_cores, use_cc):
    import concourse.bass as bass  # noqa: F401
    import concourse.mybir as mybir
    import concourse.tile as tile
    from concourse import bacc, library_config

    f32 = mybir.dt.float32
    bf16 = mybir.dt.bfloat16
    fp8 = mybir.dt.float8e4
    i16 = mybir.dt.int16
    AF = mybir.ActivationFunctionType
    ALU = mybir.AluOpType
    AX = mybir.AxisListType
    DR = mybir.MatmulPerfMode.DoubleRow

    NJ = b_loc // NB             # 4 j-tiles
    NIX = NB // 16               # idx cols per (field, j) block

    nc = bacc.Bacc(
        "TRN2",
        target_bir_lowering=False,
        debug=False,
        num_devices=n_cores,
    )

    # ---- DRAM I/O ----
    tabs = [
        nc.dram_tensor(f"tab{f}", [CAT_SIZES[f], ROWW], bf16,
                       kind="ExternalInput").ap()
        for f in range(F)
    ]
    ix_d = nc.dram_tensor("ix", [P, NJ * F * NIX], i16,
                          kind="ExternalInput").ap()
    w1q_d = nc.dram_tensor("w1q", [P, 4, 2048], fp8, kind="ExternalInput").ap()
    w2q_d = nc.dram_tensor("w2q", [P, 16, 1024], fp8, kind="ExternalInput").ap()
    w3q_d = nc.dram_tensor("w3q", [P, 8, 512], fp8, kind="ExternalInput").ap()
    w4q_d = nc.dram_tensor("w4q", [P, 4], fp8, kind="ExternalInput").ap()
    b1p_d = nc.dram_tensor("b1p", [P, 16], f32, kind="ExternalInput").ap()
    b2p_d = nc.dram_tensor("b2p", [P, 8], f32, kind="ExternalInput").ap()
    b3p_d = nc.dram_tensor("b3p", [P, 4], f32, kind="ExternalInput").ap()
    bc_d = nc.dram_tensor("bconst", [1, 1], f32, kind="ExternalInput").ap()
    sel_d = nc.dram_tensor("selc", [P, 65], bf16, kind="ExternalInput").ap()
    if use_cc:
        y_d = nc.dram_tensor("y", [1, b_loc], f32, kind="ExternalOutput").ap()
    else:
        ypre_d = nc.dram_tensor("ypre", [1, b_loc], f32,
                                kind="ExternalOutput").ap()
        gpart_d = nc.dram_tensor("gpart", [1, 1], f32,
                                 kind="ExternalOutput").ap()

    with tile.TileContext(nc) as tc:
        with (
            tc.tile_pool(name="const", bufs=1) as const,
            tc.tile_pool(name="gat", bufs=1) as gat,
            tc.tile_pool(name="act", bufs=2) as actp,
            tc.tile_pool(name="psmm", bufs=2, space="PSUM") as psum_mm,
            tc.tile_pool(name="psst", bufs=2, space="PSUM") as psum_st,
            tc.tile_pool(name="psl4", bufs=2, space="PSUM") as psum_l4,
            tc.tile_pool(name="dram", bufs=1, space="DRAM") as dram,
        ):
            nc.gpsimd.load_library(library_config.mlp)

            # ---- constants / weights (idx first: gathers depend on it) ----
            ix_sb = const.tile([P, NJ * F * NIX], i16, tag="ix_sb")
            nc.sync.dma_start(ix_sb[:], ix_d)
            w1q = const.tile([P, 4, 2048], fp8, tag="w1q")
            nc.sync.dma_start(w1q[:], w1q_d)
            b1p = const.tile([P, 16], f32, tag="b1p")
            nc.sync.dma_start(b1p[:], b1p_d)
            b2p = const.tile([P, 8], f32, tag="b2p")
            nc.sync.dma_start(b2p[:], b2p_d)
            b3p = const.tile([P, 4], f32, tag="b3p")
            nc.sync.dma_start(b3p[:], b3p_d)
            w4q = const.tile([P, 4], fp8, tag="w4q")
            nc.sync.dma_start(w4q[:], w4q_d)
            bc_sb = const.tile([1, 1], f32, tag="bc_sb")
            nc.sync.dma_start(bc_sb[:], bc_d)
            w2q = const.tile([P, 16, 1024], fp8, tag="w2q")
            nc.sync.dma_start(w2q[:], w2q_d)
            w3q = const.tile([P, 8, 512], fp8, tag="w3q")
            nc.sync.dma_start(w3q[:], w3q_d)

            # stats selector: routes block-1 partitions 0/1/2 (fc, rowsq,
            # rowsum) to output partitions 0/32/64 — engine partition slices
            # must start at a multiple of 32
            sel = const.tile([P, 65], bf16, tag="sel")
            nc.sync.dma_start(sel[:], sel_d)

            lin_sb = const.tile([1, b_loc], f32, tag="lin_sb")
            ypre_sb = const.tile([1, b_loc], f32, tag="ypre_sb")
            gacc = const.tile([1, NB], f32, tag="gacc")
            nc.vector.memset(gacc[:], 0.0)
            sv = const.tile([1, 1], f32, tag="sv")

            def ixsl(f, j):
                k = (j * F + f) * NIX
                return ix_sb[:, k:k + NIX]

            # ---- all gathers up front (pool queue; DGE ~1.5us each) ----
            G = {}
            for j in range(NJ):
                for f in range(F):
                    g = gat.tile([P, 2, NB], bf16, tag=f"g{f}_{j}",
                                 name=f"g{f}_{j}")
                    nc.gpsimd.dma_gather(
                        g[:], tabs[f], ixsl(f, j), NB, NB, ROWW,
                        transpose=True, single_packet=False,
                    )
                    G[(f, j)] = g

            # fp8 pair tiles for L1 rhs: PT[g][:, c, :] = emb of field 2g+c
            PT = {}
            for j in range(NJ):
                for g in range(2):
                    PT[(g, j)] = gat.tile([P, 2, NB], fp8, tag=f"p{g}_{j}",
                                          name=f"p{g}_{j}")

            def emit_casts(j):
                # scalar handles early tiles; gpsimd takes over once its
                # gather queue drains (it cannot touch PSUM so casts are the
                # only big work it can steal)
                for f in range(F):
                    dst = PT[(f // 2, j)][:, f % 2, :]
                    src = G[(f, j)][:, 0, :]
                    if j < 2:
                        nc.scalar.activation(dst, src, AF.Copy)
                    else:
                        nc.gpsimd.tensor_copy(dst, src)

            def act_relu(on_scalar, dst, ps_slice, bias_ap):
                if on_scalar:
                    nc.scalar.activation(dst, ps_slice, AF.Relu, bias=bias_ap)
                else:
                    nc.vector.tensor_scalar(dst, ps_slice, bias_ap, 0.0,
                                            ALU.add, ALU.max)

            # ---- stats (PE) + FM partials (DVE) per j ----
            psS = {}

            def emit_stats(j):
                ps = psum_st.tile([65, NB], f32, tag="st", name=f"st{j}")
                for f in range(F):
                    nc.tensor.matmul(
                        ps[:], lhsT=sel[:], rhs=G[(f, j)][:, 1, :],
                        start=(f == 0), stop=(f == F - 1),
                    )
                psS[j] = ps

            def emit_fm(j):
                ps = psS[j]
                jsl = slice(j * NB, (j + 1) * NB)
                # keep the lin row in SBUF so the PSUM bank can recycle
                nc.scalar.activation(lin_sb[:, jsl], ps[0:1, :], AF.Copy)
                t1 = actp.tile([1, NB], f32, tag="fmt", name=f"fmt{j}")
                nc.scalar.activation(t1[:], ps[64:65, :], AF.Square)
                nc.vector.tensor_tensor(out=t1[:], in0=t1[:],
                                        in1=ps[32:33, :], op=ALU.subtract)
                nc.vector.tensor_tensor(out=gacc[:], in0=gacc[:],
                                        in1=t1[:], op=ALU.add)

            # ---- MLP layers for one j-tile ----
            H = {}

            def emit_l1(j):
                H1 = [actp.tile([P, 2, NB], fp8, tag=f"h1_{g}",
                                name=f"h1_{g}_{j}") for g in range(8)]
                H[(1, j)] = H1
                for mt in range(16):
                    q = mt % 2
                    if q == 0:
                        ps = psum_mm.tile([P, 2, NB], f32, tag="mm",
                                          name=f"mm1_{mt}_{j}")
                    for g in range(2):
                        nc.tensor.matmul(
                            ps[:, q, :],
                            lhsT=w1q[:, 2 * g:2 * g + 2, mt * P:(mt + 1) * P],
                            rhs=PT[(g, j)][:],
                            start=(g == 0), stop=(g == 1),
                            perf_mode=DR,
                        )
                    act_relu(mt % 2 == 0, H1[mt // 2][:, mt % 2, :],
                             ps[:, q, :], b1p[:, mt:mt + 1])

            def emit_l2(j):
                H2 = [actp.tile([P, 2, NB], fp8, tag=f"h2_{g}",
                                name=f"h2_{g}_{j}") for g in range(4)]
                H[(2, j)] = H2
                H1 = H[(1, j)]
                for mt in range(8):
                    q = mt % 2
                    if q == 0:
                        ps = psum_mm.tile([P, 2, NB], f32, tag="mm",
                                          name=f"mm2_{mt}_{j}")
                    for g in range(8):
                        nc.tensor.matmul(
                            ps[:, q, :],
                            lhsT=w2q[:, 2 * g:2 * g + 2, mt * P:(mt + 1) * P],
                            rhs=H1[g][:],
                            start=(g == 0), stop=(g == 7),
                            perf_mode=DR,
                        )
                    act_relu(mt % 2 == 0, H2[mt // 2][:, mt % 2, :],
                             ps[:, q, :], b2p[:, mt:mt + 1])

            def emit_l3(j):
                H3 = [actp.tile([P, 2, NB], fp8, tag=f"h3_{g}",
                                name=f"h3_{g}_{j}") for g in range(2)]
                H[(3, j)] = H3
                H2 = H[(2, j)]
                for mt in range(4):
                    q = mt % 2
                    if q == 0:
                        ps = psum_mm.tile([P, 2, NB], f32, tag="mm",
                                          name=f"mm3_{mt}_{j}")
                    for g in range(4):
                        nc.tensor.matmul(
                            ps[:, q, :],
                            lhsT=w3q[:, 2 * g:2 * g + 2, mt * P:(mt + 1) * P],
                            rhs=H2[g][:],
                            start=(g == 0), stop=(g == 3),
                            perf_mode=DR,
                        )
                    act_relu(mt % 2 == 0, H3[mt // 2][:, mt % 2, :],
                             ps[:, q, :], b3p[:, mt:mt + 1])

            def emit_l4(j):
                jsl = slice(j * NB, (j + 1) * NB)
                H3 = H[(3, j)]
                ps4 = psum_l4.tile([1, NB], f32, tag="l4", name=f"l4_{j}")
                for kt in range(4):
                    nc.tensor.matmul(
                        ps4[:], lhsT=w4q[:, kt:kt + 1],
                        rhs=H3[kt // 2][:, kt % 2, :],
                        start=(kt == 0), stop=(kt == 3),
                    )
                # ypre = mlp_pre + lin
                nc.vector.tensor_tensor(out=ypre_sb[:, jsl], in0=ps4[:],
                                        in1=lin_sb[:, jsl], op=ALU.add)

            # ---- software-pipelined emission ----
            # scalar queue: casts j0, casts j1 (ahead of acts), acts...
            emit_casts(0)
            emit_stats(0)
            emit_fm(0)
            emit_casts(1)
            emit_l1(0)
            emit_stats(1)
            emit_fm(1)
            emit_l2(0)
            emit_casts(2)
            emit_l3(0)
            emit_l4(0)
            emit_l1(1)
            emit_stats(2)
            emit_fm(2)
            emit_casts(3)
            emit_l2(1)
            emit_l3(1)
            emit_l4(1)
            emit_l1(2)
            emit_stats(3)
            emit_fm(3)

            # FM partial -> scalar, then cross-core AllGather (hidden under
            # the remaining MLP work)
            gp = const.tile([1, 1], f32, tag="gp")
            nc.vector.reduce_sum(out=gp[:], in_=gacc[:], axis=AX.X)
            if use_cc:
                gin = dram.tile([1, 1], f32, tag="gin")
                gout = dram.tile([1, n_cores], f32, tag="gout",
                                 addr_space="Shared")
                nc.gpsimd.dma_start(gin[:], gp[:])
                nc.gpsimd.collective_compute(
                    "AllGather",
                    mybir.AluOpType.bypass,
                    replica_groups=[list(range(n_cores))],
                    ins=[gin.opt()],
                    outs=[gout.opt()],
                )
                gsb = const.tile([1, n_cores], f32, tag="gsb")
                nc.sync.dma_start(gsb[:], gout[:])
                gsum = const.tile([1, 1], f32, tag="gsum")
                nc.vector.reduce_sum(out=gsum[:], in_=gsb[:], axis=AX.X)
                # S = bias + b4 + 0.5 * sum(gparts)
                nc.scalar.activation(sv[:], gsum[:], AF.Identity,
                                     bias=bc_sb[0:1, 0:1], scale=0.5)

            emit_l2(2)
            emit_l3(2)
            emit_l4(2)
            emit_l1(3)
            emit_l2(3)
            emit_l3(3)
            emit_l4(3)

            if use_cc:
                ysb = const.tile([1, b_loc], f32, tag="ysb")
                nc.scalar.activation(ysb[:], ypre_sb[:], AF.Sigmoid,
                                     bias=sv[0:1, 0:1])
                nc.sync.dma_start(y_d, ysb[:])
            else:
                nc.sync.dma_start(ypre_d, ypre_sb[:])
                nc.sync.dma_start(gpart_d, gp[:])

    nc.compile()
    return nc


def _build_b(b_loc, n_cores):
    """Fallback phase B (no-collective mode): y = sigmoid(ypre + S)."""
    import concourse.mybir as mybir
    import concourse.tile as tile
    from concourse import bacc

    f32 = mybir.dt.float32
    AF = mybir.ActivationFunctionType
    NCH = b_loc // P

    nc = bacc.Bacc(
        "TRN2",
        target_bir_lowering=False,
        debug=False,
        num_devices=n_cores,
    )
    yin_d = nc.dram_tensor("yin", [P, NCH], f32, kind="ExternalInput").ap()
    sv_d = nc.dram_tensor("sv", [P, 1], f32, kind="ExternalInput").ap()
    y_d = nc.dram_tensor("y", [b_loc, 1], f32, kind="ExternalOutput").ap()

    with tile.TileContext(nc) as tc:
        with tc.tile_pool(name="const", bufs=1) as const:
            yin = const.tile([P, NCH], f32, tag="yin")
            nc.sync.dma_start(yin[:], yin_d)
            sv = const.tile([P, 1], f32, tag="sv")
            nc.sync.dma_start(sv[:], sv_d)
            ysb = const.tile([P, NCH], f32, tag="ysb")
            nc.scalar.activation(ysb[:], yin[:], AF.Sigmoid, bias=sv[:])
            nc.sync.dma_start(y_d.rearrange("(c p) o -> p (c o)", p=P), ysb[:])

    nc.compile()
    return nc


def _get_program(phase, b_loc, n_cores, use_cc=True):
    key = (phase, b_loc, n_cores, use_cc)
    if key not in _build_cache:
        _build_cache[key] = (
            _build_main(b_loc, n_cores, use_cc) if phase == "A"
            else _build_b(b_loc, n_cores)
        )
    return _build_cache[key]


def _wrap_idx(lin_idx):
    """[n] int -> [128, n//16] int16 dma_gather index tile (16-wrap,
    replicated for the 8 Q7 cores)."""
    n = lin_idx.shape[0]
    wrap = lin_idx.astype(np.int16).reshape(n // 16, 16).T  # [16, n//16]
    return np.ascontiguousarray(np.tile(wrap, (8, 1)))


def _prep_shared(inputs):
    """Host-side table/weight prep shared by all cores."""
    bf = ml_dtypes.bfloat16
    f8 = ml_dtypes.float8_e4m3
    emb32 = np.asarray(inputs["emb_table"], np.float32)
    emb16 = emb32.astype(bf)                                  # [T, 128]
    fc32 = np.asarray(inputs["fc"], np.float32).reshape(-1)   # [T]
    # device sees bf16 emb values; compute row stats from the rounded rows
    emb16f = emb16.astype(np.float32)
    rowsq = (emb16f * emb16f).sum(axis=1)                     # [T]
    rowsum = emb16f.sum(axis=1)                               # [T]

    sh = {}
    for f in range(F):
        sz, off = CAT_SIZES[f], int(OFFSETS_NP[f])
        tab = np.zeros((sz, ROWW), dtype=bf)
        tab[:, :EMB] = emb16[off:off + sz]
        tab[:, EMB] = fc32[off:off + sz].astype(bf)
        tab[:, EMB + 1] = rowsq[off:off + sz].astype(bf)
        tab[:, EMB + 2] = rowsum[off:off + sz].astype(bf)
        sh[f"tab{f}"] = np.ascontiguousarray(tab)

    def dr_pack(w, kgroups):
        K, M = w.shape
        w = np.asarray(w, np.float32).reshape(kgroups, 2, P, M)
        return np.ascontiguousarray(
            w.transpose(2, 0, 1, 3).reshape(P, 2 * kgroups, M).astype(f8)
        )

    sh["w1q"] = dr_pack(np.asarray(inputs["W1"]), 2)
    sh["w2q"] = dr_pack(np.asarray(inputs["W2"]), 8)
    sh["w3q"] = dr_pack(np.asarray(inputs["W3"]), 4)
    sh["w4q"] = np.ascontiguousarray(
        np.asarray(inputs["W4"], np.float32).reshape(4, P).T.astype(f8)
    )
    for name, mt in (("b1", 16), ("b2", 8), ("b3", 4)):
        sh[f"{name}p"] = np.ascontiguousarray(
            np.asarray(inputs[name], np.float32).reshape(mt, P).T
        )
    bconst = (np.asarray(inputs["bias"], np.float32).reshape(-1)[0]
              + np.asarray(inputs["b4"], np.float32).reshape(-1)[0])
    sh["bconst"] = np.full((1, 1), bconst, dtype=np.float32)
    selc = np.zeros((P, 65), dtype=bf)
    for s in range(3):
        selc[s, 32 * s] = 1.0
    sh["selc"] = np.ascontiguousarray(selc)
    return sh


def _pack_ix(xs):
    """Per-core [b_loc, F] raw ids -> [128, NJ*F*NIX] int16, (j, f)-block
    order matching the kernel's ixsl()."""
    b_loc = xs.shape[0]
    NJ = b_loc // NB
    cols = []
    for j in range(NJ):
        for f in range(F):
            cols.append(_wrap_idx(xs[j * NB:(j + 1) * NB, f]))
    return np.ascontiguousarray(np.concatenate(cols, axis=1))


def kernel(**inputs) -> np.ndarray:
    from concourse.bass_utils import run_bass_kernel_spmd

    n_cores = N_CORES
    b_loc = B // n_cores
    cores = list(range(n_cores))
    trace = bool(int(os.environ.get("KERNEL_TRACE", "0")))
    use_cc = not bool(int(os.environ.get("KERNEL_NO_CC", "0")))

    x_int = np.asarray(inputs["x"], np.float32).astype(np.int32)  # [B, F]
    shared = _prep_shared(inputs)

    ncA = _get_program("A", b_loc, n_cores, use_cc)
    in_maps = []
    for c in range(n_cores):
        m = dict(shared)
        m["ix"] = _pack_ix(x_int[c * b_loc:(c + 1) * b_loc])
        in_maps.append(m)
    resA = run_bass_kernel_spmd(ncA, in_maps, core_ids=cores, trace=trace)

    if use_cc:
        kernel._last_results = (resA,)
        kernel._last_exec_ns = resA.exec_time_ns
        kernel._last_exec_parts = (resA.exec_time_ns,)
        out = np.concatenate(
            [np.asarray(r["y"], np.float32).reshape(b_loc) for r in resA.results]
        )
        return out.reshape(B, 1).astype(np.float32)

    # ---- fallback: host-side reduction + tiny phase B ----
    g = np.float32(0.0)
    for r in resA.results:
        g = np.float32(g + np.float32(r["gpart"][0, 0]))
    S = np.float32(shared["bconst"][0, 0] + 0.5 * g)

    ncB = _get_program("B", b_loc, n_cores)
    sv = np.full((P, 1), S, dtype=np.float32)
    NCH = b_loc // P
    in_maps_b = []
    for c in range(n_cores):
        ypre = np.asarray(resA.results[c]["ypre"], np.float32).reshape(b_loc)
        in_maps_b.append({
            "yin": np.ascontiguousarray(ypre.reshape(NCH, P).T),
            "sv": sv,
        })
    resB = run_bass_kernel_spmd(ncB, in_maps_b, core_ids=cores, trace=trace)

    kernel._last_results = (resA, resB)
    a_ns, b_ns = resA.exec_time_ns, resB.exec_time_ns
    kernel._last_exec_ns = (
        (a_ns or 0) + (b_ns or 0) if (a_ns is not None or b_ns is not None)
        else None
    )
    kernel._last_exec_parts = (a_ns, b_ns)
    out = np.concatenate([r["y"] for r in resB.results], axis=0)
    return out.astype(np.float32)
